# revision 18
# baseline (speedup 1.0000x reference)
"""GPT-2 causal attention block (S=4096, D=768, H=12) on 8 TRN2 NeuronCores.

Sharding: queries interleaved mod-8 (core c owns q = 8*t + c) -> every core
runs the identical SPMD graph (uniform causal work), per-core differences are
input data only (per-core causal masks, output row mapping).
K/V projection is feature-sharded (96 of 768+768 K/V features per core over
all 4096 keys), followed by ONE AllGather; everything else is local.

Device algorithm per core (bf16 compute, f32 accumulate):
  A: K^T local [96,4096], V local [4096,96], Q^T local [768,512] (strided cols)
  B: AllGather K^T||V  (1.5MB/rank, bf16)
  C: load K^T full [768,4096] to SBUF
  D: per q-tile T (128 logical rows), per group g of 8 k-chunks:
     scores S^T [128k,128q] per chunk -> PSUM staging [128,1024]
     one wide exp -> P^T bf16 SBUF; causal mask multiply on diagonal groups;
     den partials via DVE reduce; PV matmuls (V chunk as lhsT, head-pair
     col-packed) -> O^T accum
  E: den partition-reduce (ones matmul), reciprocal, broadcast matmul,
     normalize O^T -> A^T
  F: c_proj (A^T as lhsT) + bias via K=1 ones-matmul -> out [512,768] f32
"""
import numpy as np
import ml_dtypes

import concourse.bass as bass
import concourse.bacc as bacc
import concourse.mybir as mybir
import concourse.tile as tile
from concourse.bass_utils import run_bass_kernel_spmd

BF16 = mybir.dt.bfloat16
F32 = mybir.dt.float32
AF = mybir.ActivationFunctionType

S = 4096          # sequence
D = 768           # model dim
H = 12            # heads
HD = 64           # head dim
NC = 8            # cores
QL = S // NC      # 512 queries per core
NT = QL // 128    # 4 q-tiles of 128 logical rows
NKC = S // 128    # 32 k-chunks
KVF = 2 * D // NC  # 192 kv features per core (96 K + 96 V)
KF = KVF // 2      # 96

TRACE = False  # test.py sets True for neuron-profile timing

_CACHE = {}


def build_bass():
    nc = bacc.Bacc(
        "TRN2", target_bir_lowering=False, debug=False, num_devices=NC
    )

    # ---- DRAM parameters (per-core inputs; all bf16 except out) ----
    tok_t = nc.declare_dram_parameter("tok_t", [D, S], BF16, isOutput=False)
    tok_q = nc.declare_dram_parameter("tok_q", [D, QL], BF16, isOutput=False)
    w_q = nc.declare_dram_parameter("w_q", [D, D], BF16, isOutput=False)
    b_q = nc.declare_dram_parameter("b_q", [128, 6], F32, isOutput=False)
    w_kv = nc.declare_dram_parameter("w_kv", [D, KVF], BF16, isOutput=False)
    b_k = nc.declare_dram_parameter("b_k", [128, 1], F32, isOutput=False)
    b_v = nc.declare_dram_parameter("b_v", [1, KF], BF16, isOutput=False)
    w_p = nc.declare_dram_parameter("w_p", [D, D], BF16, isOutput=False)
    b_p = nc.declare_dram_parameter("b_p", [1, D], BF16, isOutput=False)
    maskp = nc.declare_dram_parameter("maskp", [128, 8, 1024], BF16, isOutput=False)
    onesBPp = nc.declare_dram_parameter("onesBPp", [128, 64], F32, isOutput=False)
    ones128p = nc.declare_dram_parameter("ones128p", [128, 1], F32, isOutput=False)
    ones1p = nc.declare_dram_parameter("ones1p", [1, 128], BF16, isOutput=False)
    out = nc.declare_dram_parameter("out", [QL, D], F32, isOutput=True)
    dbg1 = nc.declare_dram_parameter("dbg1", [128, 65 * H], F32, isOutput=True)
    dbg2 = nc.declare_dram_parameter("dbg2", [128, H, 256], F32, isOutput=True)

    # internal DRAM: collective bounce buffers
    KSZ = KF * S          # 96*4096 K^T elements
    VSZ = S * KF          # 4096*96 V elements
    kv_bounce = nc.dram_tensor("kv_bounce", [KSZ + VSZ], BF16)
    kv_gath = nc.dram_tensor("kv_gath", [NC, KSZ + VSZ], BF16, addr_space="Shared")
    v65 = nc.dram_tensor("v65", [S, 65 * H], BF16)

    kb_k = kv_bounce[0:KSZ].rearrange("(f s) -> f s", f=KF)       # [96, 4096]
    kb_v = kv_bounce[KSZ:KSZ + VSZ].rearrange("(s f) -> s f", s=S)  # [4096, 96]

    with tile.TileContext(nc) as tc:
        with (
            tc.tile_pool(name="qt", bufs=1) as qt_pool,
            tc.tile_pool(name="kt", bufs=1) as kt_pool,
            tc.tile_pool(name="at", bufs=1) as at_pool,
            tc.tile_pool(name="const", bufs=1) as const_pool,
            tc.tile_pool(name="oacc", bufs=2) as oacc_pool,
            tc.tile_pool(name="den", bufs=2) as den_pool,
            tc.tile_pool(name="recip", bufs=3) as recip_pool,
            tc.tile_pool(name="psStage", bufs=2, space="PSUM") as psS,  # [128,1024] 2 banks
            tc.tile_pool(name="psPV", bufs=3, space="PSUM") as psPV,    # 1 bank each
        ):
            # ---------- persistent SBUF ----------
            qt = qt_pool.tile([128, 6, QL], BF16)      # Q^T  [feat, q]
            kt = kt_pool.tile([128, 6, S], BF16)       # K^T full [feat, k]
            aT = at_pool.tile([128, 6, QL], BF16)      # normalized attn out^T
            mask_sb = const_pool.tile([128, 8, 1024], BF16)
            onesBP = const_pool.tile([128, 64], F32)
            ones128 = const_pool.tile([128, 1], F32)
            ones1 = const_pool.tile([1, 128], BF16)
            bq_sb = const_pool.tile([128, 6], F32)
            bk_sb = const_pool.tile([128, 1], F32)
            bv_sb = const_pool.tile([1, KF], BF16)
            bp_sb = const_pool.tile([1, D], BF16)
            wp_sb = const_pool.tile([128, 6, D], BF16)

            nc.sync.dma_start(out=mask_sb[:], in_=maskp[:])
            nc.sync.dma_start(out=onesBP[:], in_=onesBPp[:])
            nc.sync.dma_start(out=ones128[:], in_=ones128p[:])
            nc.sync.dma_start(out=ones1[:], in_=ones1p[:])
            nc.sync.dma_start(out=bq_sb[:], in_=b_q[:])
            nc.sync.dma_start(out=bk_sb[:], in_=b_k[:])
            nc.sync.dma_start(out=bv_sb[:], in_=b_v[:])
            nc.sync.dma_start(out=bp_sb[:], in_=b_p[:])
            nc.sync.dma_start(
                out=wp_sb[:], in_=w_p.rearrange("(dc p) e -> p dc e", p=128)
            )

            # DVE pre-touch of DVE-read consts: TensorScalar/TensorTensor ISA
            # structs carry only ONE sync wait, so the DMA deps must already
            # be covered by the DVE engine clock before first real use.
            warm = const_pool.tile([128, 4], F32)
            nc.vector.tensor_copy(warm[:, 0:1], bk_sb[:, 0:1])
            nc.vector.tensor_copy(warm[:, 1:2], bq_sb[:, 0:1])
            nc.vector.tensor_copy(warm[:, 2:3], mask_sb[:, 0, 0:1])

            # ---------- phase A: projections ----------
            with (
                tc.tile_pool(name="tokt", bufs=1) as tokt_pool,
                tc.tile_pool(name="wA", bufs=1) as wA_pool,
                tc.tile_pool(name="ktloc", bufs=1) as ktloc_pool,
                tc.tile_pool(name="vev", bufs=3) as vev_pool,
            ):
                tok_sb = tokt_pool.tile([128, 6, S], BF16)
                tokq_sb = tokt_pool.tile([128, 6, QL], BF16)
                wq_sb = wA_pool.tile([128, 6, D], BF16)
                wkv_sb = wA_pool.tile([128, 6, KVF], BF16)
                kt_loc = ktloc_pool.tile([128, S], BF16)

                nc.sync.dma_start(
                    out=tok_sb[:], in_=tok_t.rearrange("(dc p) s -> p dc s", p=128)
                )
                nc.sync.dma_start(
                    out=tokq_sb[:], in_=tok_q.rearrange("(dc p) s -> p dc s", p=128)
                )
                nc.sync.dma_start(
                    out=wq_sb[:], in_=w_q.rearrange("(dc p) e -> p dc e", p=128)
                )
                nc.sync.dma_start(
                    out=wkv_sb[:], in_=w_kv.rearrange("(dc p) e -> p dc e", p=128)
                )

                # A1: K^T local [96, 4096] (feature-sharded, all keys)
                for sc in range(8):
                    psw = psS.tile([128, 1024], F32, tag="stage")
                    ps = psw[:, 0:512]
                    for dc in range(6):
                        nc.tensor.matmul(
                            ps[0:KF, :],
                            lhsT=wkv_sb[:, dc, 0:KF],
                            rhs=tok_sb[:, dc, 512 * sc:512 * (sc + 1)],
                            start=(dc == 0),
                            stop=(dc == 5),
                        )
                    nc.vector.tensor_scalar_add(
                        kt_loc[0:KF, 512 * sc:512 * (sc + 1)],
                        ps[0:KF, :],
                        bk_sb[0:KF, 0:1],
                    )
                nc.gpsimd.dma_start(out=kb_k[:, :], in_=kt_loc[0:KF, :])

                # A2: V local [4096, 96] natural orientation
                for st in range(32):
                    psw = psS.tile([128, 1024], F32, tag="stage")
                    ps = psw[:, 0:512]
                    for dc in range(6):
                        nc.tensor.matmul(
                            ps[:, 0:KF],
                            lhsT=tok_sb[:, dc, 128 * st:128 * (st + 1)],
                            rhs=wkv_sb[:, dc, KF:KVF],
                            start=(dc == 0),
                            stop=False,
                        )
                    nc.tensor.matmul(
                        ps[:, 0:KF],
                        lhsT=ones1[:, :],
                        rhs=bv_sb[:, :],
                        start=False,
                        stop=True,
                    )
                    vev = vev_pool.tile([128, KF], BF16)
                    nc.vector.tensor_copy(vev[:], ps[:, 0:KF])
                    nc.gpsimd.dma_start(
                        out=kb_v[128 * st:128 * (st + 1), :], in_=vev[:]
                    )

                # B: AllGather (emitted here; Tile schedules on deps)
                nc.gpsimd.collective_compute(
                    "AllGather",
                    mybir.AluOpType.bypass,
                    replica_groups=[list(range(NC))],
                    ins=[kv_bounce.ap().opt()],
                    outs=[kv_gath.ap().opt()],
                )

                # A3: Q^T local [768, 512] from per-core tok_q input
                for jc in range(6):
                    psw = psS.tile([128, 1024], F32, tag="stage")
                    ps = psw[:, 0:512]
                    for dc in range(6):
                        nc.tensor.matmul(
                            ps[:, :],
                            lhsT=wq_sb[:, dc, 128 * jc:128 * (jc + 1)],
                            rhs=tokq_sb[:, dc, :],
                            start=(dc == 0),
                            stop=(dc == 5),
                        )
                    nc.vector.tensor_scalar_add(
                        qt[:, jc, :], ps[:, :], bq_sb[:, jc:jc + 1]
                    )

            # ---------- phase C: load K^T full ----------
            for hp in range(6):
                f0 = 128 * hp
                r0, row0 = f0 // KF, f0 % KF
                n0 = min(KF - row0, 128)
                gk0 = kv_gath[r0, 0:KSZ].rearrange("(f s) -> f s", f=KF)
                nc.gpsimd.dma_start(
                    out=kt[0:n0, hp, :], in_=gk0[row0:row0 + n0, :]
                )
                if n0 < 128:
                    gk1 = kv_gath[r0 + 1, 0:KSZ].rearrange("(f s) -> f s", f=KF)
                    nc.gpsimd.dma_start(
                        out=kt[n0:128, hp, :], in_=gk1[0:128 - n0, :]
                    )

            # ---------- phase D: attention (v3) ----------
            # P^T staging g-major [128, 4, 256]; den rides the PV matmul as a
            # ones column in lhsT (out row 64); per-head pvp [65, 256].
            gv_all = kv_gath[:, KSZ:KSZ + VSZ].rearrange("r (s f) -> s r f", s=S)
            with (
                tc.tile_pool(name="vres", bufs=1) as vres_pool,
                tc.tile_pool(name="phat", bufs=6) as phat_pool,
                tc.tile_pool(name="tmpo", bufs=4) as tmpo_pool,
                tc.tile_pool(name="denp", bufs=1) as denp_pool,
                tc.tile_pool(name="oacc1", bufs=1) as oacc1_pool,
            ):
                # one-time DRAM remap: gathered V [r][4096, 96] -> v65
                # [4096, 65*H] with head h V at cols [65h : 65h+64]
                for h in range(H):
                    f0 = 64 * h
                    while f0 < 64 * (h + 1):
                        r = f0 // KF
                        f1 = min(KF * (r + 1), 64 * (h + 1))
                        gvr = kv_gath[r, KSZ:KSZ + VSZ].rearrange(
                            "(s f) -> s f", s=S
                        )
                        d0 = f0 - 64 * h
                        nc.gpsimd.dma_start(
                            out=v65[:, 65 * h + d0:65 * h + d0 + (f1 - f0)],
                            in_=gvr[:, f0 - KF * r:f1 - KF * r],
                        )
                        f0 = f1

                # v_res columns per head h: [65h:65h+64] = V_h, col 65h+64 = 1
                v_res = vres_pool.tile([128, NKC, H * 65], BF16)
                nc.vector.memset(
                    v_res[:].rearrange(
                        "p ch (h w) -> p ch h w", w=65
                    )[:, :, :, 64:65],
                    1.0,
                )
                v65v = v65.rearrange("s (h w) -> s h w", w=65)
                for ch in range(NKC):
                    nc.gpsimd.dma_start(
                        out=v_res[:, ch, :].rearrange(
                            "p (h w) -> p h w", w=65
                        )[:, :, 0:64],
                        in_=v65v[128 * ch:128 * (ch + 1), :, 0:64],
                    )

                def do_group(TT, g, oacc):
                    masked = (TT == 0) or (g >= 4)
                    mi = 4 * TT + (g if TT == 0 else g - 4)
                    for hp in range(6):
                        for hh in range(2):
                            h = 2 * hp + hh
                            p0, p1 = 64 * hh, 64 * hh + 64
                            stg = psS.tile([128, 1024], F32, tag="stage")
                            for kc in range(4):
                                ch = 4 * g + kc
                                nc.tensor.matmul(
                                    stg[:, 256 * kc:256 * (kc + 1)],
                                    lhsT=kt[p0:p1, hp, 128 * ch:128 * (ch + 1)],
                                    rhs=qt[p0:p1, hp, 256 * TT:256 * (TT + 1)],
                                    start=True,
                                    stop=True,
                                )
                            phat = phat_pool.tile([128, 4, 256], BF16, tag="phat")
                            nc.scalar.activation(
                                phat[:].rearrange("p g q -> p (g q)"),
                                stg[:, :],
                                AF.Exp,
                            )
                            if masked:
                                nc.vector.tensor_mul(
                                    phat[:].rearrange("p g q -> p (g q)"),
                                    phat[:].rearrange("p g q -> p (g q)"),
                                    mask_sb[:, mi, :],
                                )
                            pvp = psPV.tile([128, 256], F32, tag="pv")
                            for kc in range(4):
                                ch = 4 * g + kc
                                nc.tensor.matmul(
                                    pvp[0:65, :],
                                    lhsT=v_res[:, ch, 65 * h:65 * h + 65],
                                    rhs=phat[:, kc, :],
                                    start=(kc == 0),
                                    stop=(kc == 3),
                                )
                            if g == 0:
                                nc.vector.tensor_copy(
                                    oacc[0:65, h, :], pvp[0:65, :]
                                )
                            else:
                                nc.vector.tensor_add(
                                    oacc[0:65, h, :], oacc[0:65, h, :],
                                    pvp[0:65, :],
                                )

                def do_norm(TT, oacc):
                    # den rows (partition 64) -> partition 0 via DMA, recip,
                    # then per-head broadcast matmul + normalize multiply.
                    denrow = denp_pool.tile([1, H, 256], F32, tag="denrow")
                    nc.gpsimd.dma_start(out=denrow[:], in_=oacc[64:65, :, :])
                    rrow = denp_pool.tile([1, H, 256], F32, tag="rrow")
                    nc.vector.reciprocal_approx_fast(
                        rrow[:].rearrange("p h q -> p (h q)"),
                        denrow[:].rearrange("p h q -> p (h q)"),
                    )
                    for hp in range(6):
                        for hh in range(2):
                            h = 2 * hp + hh
                            bc = psPV.tile([128, 256], F32, tag="pv")
                            nc.tensor.matmul(
                                bc[0:64, :],
                                lhsT=onesBP[0:1, :],
                                rhs=rrow[:, h, :],
                                start=True,
                                stop=True,
                            )
                            if hh == 0:
                                nc.vector.tensor_mul(
                                    aT[0:64, hp, 256 * TT:256 * (TT + 1)],
                                    oacc[0:64, h, :],
                                    bc[0:64, :],
                                )
                            else:
                                tmp = tmpo_pool.tile([64, 256], BF16, tag="tmpo")
                                nc.vector.tensor_mul(
                                    tmp[:, :], oacc[0:64, h, :], bc[0:64, :]
                                )
                                nc.gpsimd.dma_start(
                                    out=aT[64:128, hp, 256 * TT:256 * (TT + 1)],
                                    in_=tmp[:, :],
                                )

                for TT in range(2):
                    oacc = oacc1_pool.tile([128, H, 256], F32, tag="oacc")
                    for g in range(4 * (TT + 1)):
                        do_group(TT, g, oacc)
                    if TT == 0:
                        nc.gpsimd.dma_start(out=dbg2[:, :, :], in_=oacc[:])
                    do_norm(TT, oacc)

            # ---------- phase F: c_proj ----------
            with tc.tile_pool(name="oev", bufs=3) as oev_pool:
                for st in range(4):
                    for ec, ew in ((0, 512), (1, 256)):
                        psw = psS.tile([128, 1024], F32, tag="stage")
                        ps = psw[:, 0:512]
                        for dc in range(6):
                            nc.tensor.matmul(
                                ps[:, 0:ew],
                                lhsT=aT[:, dc, 128 * st:128 * (st + 1)],
                                rhs=wp_sb[:, dc, 512 * ec:512 * ec + ew],
                                start=(dc == 0),
                                stop=False,
                            )
                        nc.tensor.matmul(
                            ps[:, 0:ew],
                            lhsT=ones1[:, :],
                            rhs=bp_sb[:, 512 * ec:512 * ec + ew],
                            start=False,
                            stop=True,
                        )
                        oev = oev_pool.tile([128, 512], F32, tag="oev")
                        nc.vector.tensor_copy(oev[:, 0:ew], ps[:, 0:ew])
                        nc.gpsimd.dma_start(
                            out=out[128 * st:128 * (st + 1), 512 * ec:512 * ec + ew],
                            in_=oev[:, 0:ew],
                        )
    nc.compile()
    return nc


def make_inputs_v2(tokens, c_attn_weight, c_attn_bias, c_proj_weight, c_proj_bias):
    bf = ml_dtypes.bfloat16
    tokens = np.asarray(tokens, np.float32)
    w = np.asarray(c_attn_weight, np.float32)
    b = np.asarray(c_attn_bias, np.float32)
    wp = np.asarray(c_proj_weight, np.float32)
    bp = np.asarray(c_proj_bias, np.float32)

    scale = 1.0 / np.sqrt(HD)
    tok_t_full = np.ascontiguousarray(tokens.T).astype(bf)          # [768, 4096]
    w_q = (w[:, 0:D] * scale).astype(bf)
    b_q = (b[0:D] * scale).reshape(6, 128).T.copy().astype(np.float32)  # [128, 6]
    w_p = wp.astype(bf)
    b_p = bp.reshape(1, D).astype(bf)

    onesBP = np.ones((128, 64), np.float32)
    ones128 = np.ones((128, 1), np.float32)
    ones1 = np.ones((1, 128), bf)

    in_maps = []
    for c in range(NC):
        qcols = np.arange(QL) * NC + c
        tok_q = np.ascontiguousarray(tokens.T[:, qcols]).astype(bf)  # [768, 512]

        kf = slice(D + KF * c, D + KF * (c + 1))
        vf = slice(2 * D + KF * c, 2 * D + KF * (c + 1))
        w_kv = np.concatenate([w[:, kf], w[:, vf]], axis=1).astype(bf)  # [768,192]
        b_k = np.zeros((128, 1), np.float32)
        b_k[0:KF, 0] = b[kf]
        b_v = b[vf].reshape(1, KF).astype(bf)

        # mask[i, m, kc*256 + j] (g-major, matching phat [p, g, q]):
        # m = 4*TT + g_rel; chunk ch = 4*g_abs + kc; q_abs = 2048*TT + 8*j + c
        mask = np.zeros((128, 8, 1024), np.float32)
        i = np.arange(128).reshape(128, 1, 1)
        kc = np.arange(4).reshape(1, 4, 1)
        j = np.arange(256).reshape(1, 1, 256)
        for TT in range(2):
            for gr in range(4):
                g_abs = gr if TT == 0 else gr + 4
                ch = 4 * g_abs + kc
                vis = (128 * ch + i) <= (2048 * TT + 8 * j + c)
                mask[:, 4 * TT + gr, :] = vis.reshape(128, 1024)
        mask = mask.astype(bf)

        in_maps.append({
            "tok_t": tok_t_full,
            "tok_q": tok_q,
            "w_q": w_q,
            "b_q": b_q,
            "w_kv": w_kv,
            "b_k": b_k,
            "b_v": b_v,
            "w_p": w_p,
            "b_p": b_p,
            "maskp": mask,
            "onesBPp": onesBP,
            "ones128p": ones128,
            "ones1p": ones1,
        })
    return in_maps


def kernel(tokens, attn_bias, c_attn_weight, c_attn_bias, c_proj_weight,
           c_proj_bias):
    if "nc" not in _CACHE:
        _CACHE["nc"] = build_bass()
    nc = _CACHE["nc"]
    in_maps = make_inputs_v2(
        tokens, c_attn_weight, c_attn_bias, c_proj_weight, c_proj_bias
    )
    res = run_bass_kernel_spmd(nc, in_maps, list(range(NC)), trace=TRACE)
    _CACHE["last_result"] = res
    out = np.zeros((S, D), np.float32)
    for c in range(NC):
        out[np.arange(QL) * NC + c] = res.results[c]["out"]
    return out
